# revision 10
# baseline (speedup 1.0000x reference)
"""Trainium2 Bass kernel for nn_DeepDDT (soft decision tree, 16 leaves).

Self-contained: takes FULL unsharded inputs, shards batch across 8 NeuronCores
(pure data parallel), runs a Bass/Tile kernel per core, gathers full output.

Algorithmic restructuring vs the reference:
  - calc matmuls for nodes 7..14 are dead (leaves don't consume outs) -> skipped
  - comp_n = -pw_n * mean_B(inp_n); all batch means are collected locally and
    combined with ONE 16KB AllReduce (the outs-chain itself is collective-free)
  - sigmoid(dist) is replaced by tanh: p = 0.5 + 0.5 * sum(T*e)/sum(e) with
    T = tanh(0.5*(pw*inp + comp)), so tanh/exp/relu all live in one ACT table set
  - softmax max-subtraction is skipped (|z| < 4, exp is safe in fp32)
  - leaf probabilities via a 4-level shuffle/multiply tree on-chip
All data is feat-major on chip ([feature_partition, batch_free]) so per-feature
scale/bias (pw, comp, attn_b, calc_b) are per-partition ACT operands.

chain>1 (used only by test.py for timing): the NEFF executes the body once
with the AllReduce, then `chain` more times inside a hardware For_i loop.
Collectives cannot replay inside a NEFF loop on this runtime (mesh desync),
so the loop iterations reuse the pre-loop AllReduce result; they still issue
the same DRAM round-trip DMAs for the reduction buffers. One launch therefore
performs chain+1 full kernel iterations; the AllReduce itself is timed
separately by test.py and added per-iteration.
"""

import numpy as np
import ml_dtypes

import concourse.bass as bass
import concourse.mybir as mybir
import concourse.tile as tile
from concourse import bacc, bass_utils

AF = mybir.ActivationFunctionType
ALU = mybir.AluOpType
F32 = mybir.dt.float32
BF16 = mybir.dt.bfloat16

N_CORES = 8
B, D, OUT, LEAF = 8192, 512, 64, 16
BS = B // N_CORES          # 1024 rows per core
F = 2 * D                  # 1024 internal-node input width
NT_X = D // 128            # 4 tiles of x features
NT_F = F // 128            # 8 tiles of concat features
N_NODES = 15               # root + 14 internal
N_CALC = 7                 # nodes with live calc matmuls (0..6)
BH = BS // 2               # 512: batch half (PSUM bank limit)

_BF = ml_dtypes.bfloat16

_CACHE = {}


def _parent(n):
    return (n - 1) // 2


def _build(debug=False, skip_attn=False, skip_tail=False, chain=0):
    assert chain == 0 or not (debug or skip_attn or skip_tail)
    nc = bacc.Bacc("TRN2", target_bir_lowering=False, debug=False,
                   num_devices=N_CORES)

    # ---------------- DRAM I/O ----------------
    x_d = nc.dram_tensor("x_fm", [NT_X, 128, BS], BF16, kind="ExternalInput")
    cw0_d = nc.dram_tensor("cw0", [NT_X, 128, D], BF16, kind="ExternalInput")
    cw_d = nc.dram_tensor("cw", [N_CALC - 1, NT_F, 128, D], BF16, kind="ExternalInput")
    aw0_d = nc.dram_tensor("aw0", [NT_X, 128, D], BF16, kind="ExternalInput")
    aw_d = nc.dram_tensor("aw", [14, NT_F, 128, F], BF16, kind="ExternalInput")
    pwh_d = nc.dram_tensor("pwh", [128, N_NODES * 8], F32, kind="ExternalInput")
    pw2_d = nc.dram_tensor("pw2", [128, N_NODES * 8], F32, kind="ExternalInput")
    ab_d = nc.dram_tensor("ab", [128, N_NODES * 8], F32, kind="ExternalInput")
    cb_d = nc.dram_tensor("cb", [128, N_CALC * 4], F32, kind="ExternalInput")
    ohe_d = nc.dram_tensor("ohe", [N_NODES, 128, 16], BF16, kind="ExternalInput")
    leaf_d = nc.dram_tensor("leaf", [LEAF, OUT], BF16, kind="ExternalInput")
    flip_d = nc.dram_tensor("flip", [32, 4], F32, kind="ExternalInput")
    out_d = nc.dram_tensor("out", [BS, OUT], F32, kind="ExternalOutput")
    if debug:
        dbg_o = nc.dram_tensor("dbg_o", [128, N_CALC * NT_X * BS], BF16,
                               kind="ExternalOutput")
        dbg_ps = nc.dram_tensor("dbg_ps", [128, 32], F32, kind="ExternalOutput")
        dbg_s = nc.dram_tensor("dbg_s", [128, 32], F32, kind="ExternalOutput")
        dbg_bias = nc.dram_tensor("dbg_bias", [128, N_NODES * 8], F32,
                                  kind="ExternalOutput")
        dbg_s1 = nc.dram_tensor("dbg_s1", [16, BS], F32, kind="ExternalOutput")
        dbg_s2 = nc.dram_tensor("dbg_s2", [16, BS], F32, kind="ExternalOutput")
        dbg_pp = nc.dram_tensor("dbg_pp", [32, BS], F32, kind="ExternalOutput")
        dbg_leaf = nc.dram_tensor("dbg_leaf", [32, BS], F32,
                                  kind="ExternalOutput")

    with tile.TileContext(nc) as tc:
        with (
            tc.tile_pool(name="const", bufs=1) as cpool,
            tc.tile_pool(name="obuf", bufs=1) as opool,
            tc.tile_pool(name="wc", bufs=2) as wcpool,
            tc.tile_pool(name="wa", bufs=2) as wapool,
            tc.tile_pool(name="ebuf", bufs=3) as epool,
            tc.tile_pool(name="tbuf", bufs=3) as tpool,
            tc.tile_pool(name="debuf", bufs=3) as depool,
            tc.tile_pool(name="misc", bufs=1) as mpool,
            tc.tile_pool(name="spsum", bufs=1, space="PSUM") as spool,
            tc.tile_pool(name="dram", bufs=1, space="DRAM") as dpool,
        ):
            # ---------- constants ----------
            x_sb = cpool.tile([128, NT_X * BS], BF16)
            for t in range(NT_X):
                nc.gpsimd.dma_start(x_sb[:, t * BS:(t + 1) * BS], x_d[t])
            pwh = cpool.tile([128, N_NODES * 8], F32)
            nc.gpsimd.dma_start(pwh[:], pwh_d[:])
            pw2 = cpool.tile([128, N_NODES * 8], F32)
            nc.gpsimd.dma_start(pw2[:], pw2_d[:])
            ab = cpool.tile([128, N_NODES * 8], F32)
            nc.gpsimd.dma_start(ab[:], ab_d[:])
            cb = cpool.tile([128, N_CALC * 4], F32)
            nc.gpsimd.dma_start(cb[:], cb_d[:])
            ohe = cpool.tile([128, N_NODES * 16], BF16)
            for n in range(N_NODES):
                nc.gpsimd.dma_start(ohe[:, n * 16:(n + 1) * 16], ohe_d[n])
            leaf_w = cpool.tile([LEAF, OUT], BF16)
            nc.gpsimd.dma_start(leaf_w[:], leaf_d[:])
            flip = cpool.tile([32, 4], F32)
            nc.gpsimd.dma_start(flip[:], flip_d[:])

            # o_sb: calc outputs for nodes 0..6, feat-major bf16
            o_sb = cpool.tile([128, N_CALC * NT_X * BS], BF16)

            def o_tile(c, t):  # feature tile t of node c's output
                base = (c * NT_X + t) * BS
                return o_sb[:, base:base + BS]

            def inp_tile(n, t):
                """feature tile t of node n's prob/attn input (feat-major)."""
                if n == 0:
                    return x_sb[:, t * BS:(t + 1) * BS]
                if t < NT_X:
                    return o_tile(_parent(n), t)
                return x_sb[:, (t - NT_X) * BS:(t - NT_X + 1) * BS]

            # partial sums tile: cols 0..3 = x tiles, 4+c*4+m = o_c tile m
            ps = mpool.tile([128, 32], F32, tag="ps")

            # s1/s2 accumulators (rows 0..14 = nodes)
            s1 = spool.tile([16, BS], F32, space="PSUM")
            s2 = spool.tile([16, BS], F32, space="PSUM")

            # collective DRAM round-trip buffers (shared by all reps)
            cc_in = dpool.tile([128, 32], F32)
            cc_out = dpool.tile([128, 32], F32)

            def emit_body(rep, with_cc):
                with tc.tile_pool(name=f"zpsum{rep}", bufs=2,
                                  space="PSUM") as zpool:
                    # ================= CALC PHASE =================
                    for t in range(NT_X):
                        nc.vector.reduce_sum(
                            ps[:, t:t + 1], x_sb[:, t * BS:(t + 1) * BS],
                            axis=mybir.AxisListType.X)
                    for c in range(N_CALC):
                        ntk = NT_X if c == 0 else NT_F
                        wct = wcpool.tile([128, NT_F * D], BF16, tag="wc")
                        for k in range(ntk):
                            src = cw0_d[k] if c == 0 else cw_d[c - 1, k]
                            nc.gpsimd.dma_start(wct[:, k * D:(k + 1) * D], src)
                        for m in range(NT_X):  # output feature tiles (D=512)
                            zp = zpool.tile([128, BS], F32, tag="zp")
                            for k in range(ntk):
                                lhs = wct[:, k * D + m * 128: k * D + (m + 1) * 128]
                                for h in range(2):
                                    rhs = (x_sb[:, k * BS + h * BH: k * BS + h * BH + BH]
                                           if c == 0 else
                                           inp_tile(c, k)[:, h * BH: h * BH + BH])
                                    nc.tensor.matmul(
                                        zp[:, h * BH: h * BH + BH], lhs, rhs,
                                        start=(k == 0), stop=(k == ntk - 1))
                            nc.scalar.activation(
                                o_tile(c, m), zp[:], AF.Relu,
                                bias=cb[:, c * 4 + m: c * 4 + m + 1])
                            nc.vector.reduce_sum(
                                ps[:, 4 + c * 4 + m: 5 + c * 4 + m], o_tile(c, m),
                                axis=mybir.AxisListType.X)

                    # ============ ALLREDUCE (one, 16KB) ============
                    # Loop reps (with_cc=False) reuse the pre-loop AllReduce
                    # result in cc_out but still do both DRAM DMAs.
                    nc.gpsimd.dma_start(cc_in[:], ps[:])
                    if with_cc:
                        nc.gpsimd.collective_compute(
                            "AllReduce", ALU.add,
                            replica_groups=[list(range(N_CORES))],
                            ins=[cc_in[:]], outs=[cc_out[:]])
                    s_sb = mpool.tile([128, 32], F32, tag="s_sb")
                    nc.gpsimd.dma_start(s_sb[:], cc_out[:])

                    # comp bias: bias_all = pw2 (.) gathered sums
                    tmp = mpool.tile([128, N_NODES * 8], F32, tag="tmp")
                    nc.vector.memset(tmp[:], 0.0)
                    nc.vector.tensor_copy(tmp[:, 0:4], s_sb[:, 0:4])  # root <- x
                    for n in range(1, N_NODES):
                        pc = 4 + _parent(n) * 4
                        nc.vector.tensor_copy(tmp[:, n * 8:n * 8 + 4],
                                              s_sb[:, pc:pc + 4])
                        nc.vector.tensor_copy(tmp[:, n * 8 + 4:n * 8 + 8],
                                              s_sb[:, 0:4])
                    bias_all = mpool.tile([128, N_NODES * 8], F32, tag="bias")
                    nc.vector.tensor_mul(bias_all[:], pw2[:], tmp[:])
                    if debug:
                        nc.gpsimd.dma_start(dbg_o[:], o_sb[:])
                        nc.gpsimd.dma_start(dbg_ps[:], ps[:])
                        nc.gpsimd.dma_start(dbg_s[:], s_sb[:])
                        nc.gpsimd.dma_start(dbg_bias[:], bias_all[:])

                    # ================= ATTN PHASE =================
                    # Per node: accumulate e and T*e across the m feature
                    # tiles on DVE (bf16), then reduce partitions with ONE
                    # one-hot matmul per (value, batch-half) instead of one
                    # per (m, value, batch-half): 60 PE matmuls instead of 464.
                    # The one-hot reduction for node n is emitted AFTER node
                    # n+1's dense matmuls so the PE queue never waits on the
                    # ACT/DVE accumulation chain (software pipelining).
                    def emit_onehot(n, e_acc, de_acc):
                        oh = ohe[:, n * 16:(n + 1) * 16]
                        first = (n == 0)
                        last = (n == N_NODES - 1)
                        for h in range(2):
                            nc.tensor.matmul(
                                s1[:, h * BH: h * BH + BH], oh,
                                e_acc[:, h * BH: h * BH + BH],
                                start=first, stop=last, skip_group_check=True)
                            nc.tensor.matmul(
                                s2[:, h * BH: h * BH + BH], oh,
                                de_acc[:, h * BH: h * BH + BH],
                                start=first, stop=last, skip_group_check=True)

                    pending = None
                    for n in ([] if skip_attn else range(N_NODES)):
                        ntf = NT_X if n == 0 else NT_F
                        wat = wapool.tile([128, NT_F * F], BF16, tag="wa")
                        wid = D if n == 0 else F
                        for k in range(ntf):
                            src = aw0_d[k] if n == 0 else aw_d[n - 1, k]
                            nc.gpsimd.dma_start(wat[:, k * wid:(k + 1) * wid], src)
                        e_acc = epool.tile([128, BS], BF16, tag="eacc", bufs=2)
                        de_acc = depool.tile([128, BS], BF16, tag="deacc", bufs=2)
                        for m in range(ntf):
                            zp = zpool.tile([128, BS], F32, tag="zp")
                            for k in range(ntf):
                                lhs = wat[:, k * wid + m * 128: k * wid + (m + 1) * 128]
                                for h in range(2):
                                    rhs = inp_tile(n, k)[:, h * BH: h * BH + BH]
                                    nc.tensor.matmul(
                                        zp[:, h * BH: h * BH + BH], lhs, rhs,
                                        start=(k == 0), stop=(k == ntf - 1))
                            col = n * 8 + m
                            e_m = (e_acc if m == 0 else
                                   epool.tile([128, BS], BF16, tag="e"))
                            nc.scalar.activation(e_m[:], zp[:], AF.Exp,
                                                 bias=ab[:, col:col + 1])
                            t_m = tpool.tile([128, BS], BF16, tag="t")
                            nc.scalar.activation(
                                t_m[:], inp_tile(n, m), AF.Tanh,
                                bias=bias_all[:, col:col + 1],
                                scale=pwh[:, col:col + 1])
                            if m == 0:
                                nc.vector.tensor_mul(de_acc[:], t_m[:], e_m[:])
                            else:
                                de_m = depool.tile([128, BS], BF16, tag="de")
                                nc.vector.tensor_mul(de_m[:], t_m[:], e_m[:])
                                nc.vector.tensor_add(e_acc[:], e_acc[:], e_m[:])
                                nc.vector.tensor_add(de_acc[:], de_acc[:], de_m[:])
                        if pending is not None:
                            emit_onehot(*pending)
                        pending = (n, e_acc, de_acc)
                    if pending is not None:
                        emit_onehot(*pending)

                # ================= TAIL =================
                # Gate selection: row pattern [q_n, p_n] alternating is obtained
                # by shuffling p rows then flipping alternate rows via per-
                # partition affine (q = 1 - p); avoids any non-32-aligned
                # partition access.
                if skip_attn:
                    zt = mpool.tile([128, OUT], F32, tag="zt")
                    nc.vector.memset(zt[:], 0.0)
                    for bt in range(BS // 128):
                        nc.gpsimd.dma_start(out_d[bt * 128:(bt + 1) * 128, :], zt[:])
                    zt16 = mpool.tile([16, BS], F32, tag="zt16")
                    nc.vector.memset(zt16[:], 0.0)
                    nc.vector.tensor_copy(s1[:], zt16[:])
                    nc.vector.tensor_copy(s2[:], zt16[:])
                if skip_tail:
                    if debug:
                        s1c = mpool.tile([16, BS], F32, tag="s1c")
                        nc.vector.tensor_copy(s1c[:], s1[:])
                        nc.gpsimd.dma_start(dbg_s1[:], s1c[:])
                        s2c = mpool.tile([16, BS], F32, tag="s2c")
                        nc.vector.tensor_copy(s2c[:], s2[:])
                        nc.gpsimd.dma_start(dbg_s2[:], s2c[:])
                        zt32 = mpool.tile([32, BS], F32, tag="zt32")
                        nc.vector.memset(zt32[:], 0.0)
                        nc.gpsimd.dma_start(dbg_pp[:], zt32[:])
                        nc.gpsimd.dma_start(dbg_leaf[:], zt32[:])
                    if not skip_attn:
                        zt = mpool.tile([128, OUT], F32, tag="zt")
                        nc.vector.memset(zt[:], 0.0)
                        for bt in range(BS // 128):
                            nc.gpsimd.dma_start(out_d[bt * 128:(bt + 1) * 128, :],
                                                zt[:])
                with tc.tile_pool(name=f"tpsum{rep}", bufs=2,
                                  space="PSUM") as tpsum:
                  if not skip_tail:
                    rec = mpool.tile([16, BS], F32, tag="rec")
                    nc.vector.reciprocal(rec[:], s1[:])
                    rat = mpool.tile([16, BS], F32, tag="rat")
                    nc.vector.tensor_mul(rat[:], s2[:], rec[:])
                    pp = mpool.tile([32, BS], F32, tag="pp")
                    nc.vector.tensor_scalar(pp[0:16, :], rat[:], 0.5, 0.5,
                                            ALU.mult, ALU.add)

                    def shuf(dst, src, mask):
                        mask = mask + [0] * (32 - len(mask))
                        nc.vector.stream_shuffle(dst[:], src[:], mask=mask)

                    # flip coefficient columns: 0=alt_a 1=alt_b 2=pair_a 3=pair_b
                    e2 = mpool.tile([32, BS], F32, tag="e2")
                    s2v = mpool.tile([32, BS], F32, tag="s2v")
                    shuf(e2, pp, [0, 0, 0, 0])
                    nc.vector.tensor_scalar(e2[0:4, :], e2[0:4, :],
                                            flip[0:4, 2:3], flip[0:4, 3:4],
                                            ALU.mult, ALU.add)
                    shuf(s2v, pp, [1, 1, 2, 2])
                    nc.vector.tensor_scalar(s2v[0:4, :], s2v[0:4, :],
                                            flip[0:4, 0:1], flip[0:4, 1:2],
                                            ALU.mult, ALU.add)
                    l2 = mpool.tile([32, BS], F32, tag="l2")
                    nc.vector.tensor_mul(l2[0:4, :], e2[0:4, :], s2v[0:4, :])
                    e3 = mpool.tile([32, BS], F32, tag="e3")
                    s3v = mpool.tile([32, BS], F32, tag="s3v")
                    shuf(e3, l2, [0, 0, 1, 1, 2, 2, 3, 3])
                    shuf(s3v, pp, [3, 3, 4, 4, 5, 5, 6, 6])
                    nc.vector.tensor_scalar(s3v[0:8, :], s3v[0:8, :],
                                            flip[0:8, 0:1], flip[0:8, 1:2],
                                            ALU.mult, ALU.add)
                    l3 = mpool.tile([32, BS], F32, tag="l3")
                    nc.vector.tensor_mul(l3[0:8, :], e3[0:8, :], s3v[0:8, :])
                    e4 = mpool.tile([32, BS], F32, tag="e4")
                    s4v = mpool.tile([32, BS], F32, tag="s4v")
                    shuf(e4, l3, [i // 2 for i in range(16)])
                    shuf(s4v, pp, sum([[7 + i, 7 + i] for i in range(8)], []))
                    nc.vector.tensor_scalar(s4v[0:16, :], s4v[0:16, :],
                                            flip[0:16, 0:1], flip[0:16, 1:2],
                                            ALU.mult, ALU.add)
                    leaf_p = mpool.tile([32, BS], BF16, tag="leaf_p")
                    nc.vector.tensor_mul(leaf_p[0:16, :], e4[0:16, :], s4v[0:16, :])
                    if debug:
                        s1c = mpool.tile([16, BS], F32, tag="s1c")
                        nc.vector.tensor_copy(s1c[:], s1[:])
                        nc.gpsimd.dma_start(dbg_s1[:], s1c[:])
                        s2c = mpool.tile([16, BS], F32, tag="s2c")
                        nc.vector.tensor_copy(s2c[:], s2[:])
                        nc.gpsimd.dma_start(dbg_s2[:], s2c[:])
                        nc.gpsimd.dma_start(dbg_pp[:], pp[:])
                        lpc = mpool.tile([32, BS], F32, tag="lpc")
                        nc.vector.memset(lpc[:], 0.0)
                        nc.vector.tensor_copy(lpc[0:16, :], e4[0:16, :])
                        nc.vector.tensor_mul(lpc[0:16, :], lpc[0:16, :], s4v[0:16, :])
                        nc.gpsimd.dma_start(dbg_leaf[:], lpc[:])

                    # actions + softmax, batch-major
                    for bt in range(BS // 128):
                        ap = tpsum.tile([128, OUT], F32, tag="act")
                        nc.tensor.matmul(ap[:], leaf_p[0:16, bt * 128:(bt + 1) * 128],
                                         leaf_w[:], start=True, stop=True)
                        ea = mpool.tile([128, OUT], F32, tag="ea", bufs=2)
                        nc.scalar.activation(ea[:], ap[:], AF.Exp)
                        ssum = mpool.tile([128, 1], F32, tag="ssum", bufs=2)
                        nc.vector.reduce_sum(ssum[:], ea[:], axis=mybir.AxisListType.X)
                        rs = mpool.tile([128, 1], F32, tag="rs", bufs=2)
                        nc.vector.reciprocal(rs[:], ssum[:])
                        ot = mpool.tile([128, OUT], F32, tag="ot", bufs=2)
                        nc.vector.tensor_scalar(ot[:], ea[:], rs[:], None, ALU.mult)
                        nc.gpsimd.dma_start(out_d[bt * 128:(bt + 1) * 128, :], ot[:])

            emit_body(0, True)
            if chain > 0:
                with tc.For_i(0, chain, staggered_reset=True):
                    emit_body(1, False)

    nc.compile()
    return nc


def _pack_inputs(inputs):
    """Host-side packing: transposes, bf16 casts, per-node packed vectors."""
    x = np.asarray(inputs["x"], np.float32)
    cW0 = np.asarray(inputs["calc_W0"], np.float32)
    cb0 = np.asarray(inputs["calc_b0"], np.float32)
    pw0 = np.asarray(inputs["prob_w0"], np.float32)
    aW0 = np.asarray(inputs["attn_W0"], np.float32)
    ab0 = np.asarray(inputs["attn_b0"], np.float32)
    cW = np.asarray(inputs["calc_W"], np.float32)
    cb = np.asarray(inputs["calc_b"], np.float32)
    pw = np.asarray(inputs["prob_w"], np.float32)
    aW = np.asarray(inputs["attn_W"], np.float32)
    ab_i = np.asarray(inputs["attn_b"], np.float32)
    leaf_out = np.asarray(inputs["leaf_out"], np.float32)

    # x feat-major per core: [NT_X, 128, BS]
    x_fm = np.ascontiguousarray(x.T).astype(_BF)          # [D, B]
    x_cores = [np.ascontiguousarray(
        x_fm[:, c * BS:(c + 1) * BS].reshape(NT_X, 128, BS))
        for c in range(N_CORES)]

    # weights: lhsT = W.T, split into k-tiles [K/128, 128, M]
    def kt(wT, m):  # wT [K, M]
        return np.ascontiguousarray(wT.reshape(-1, 128, m)).astype(_BF)

    cw0 = kt(cW0.T, D)                                    # [4,128,512]
    cw = np.stack([kt(cW[i].T, D) for i in range(6)])     # [6,8,128,512]
    aw0 = kt(aW0.T, D)                                    # [4,128,512]
    aw = np.stack([kt(aW[i].T, F) for i in range(14)])    # [14,8,128,1024]

    def pack_cols(vecs, ncols):
        """vecs: list of (per-node 1D arrays); -> [128, ncols]"""
        out = np.zeros((128, ncols), np.float32)
        for n, v in enumerate(vecs):
            ntv = v.shape[0] // 128
            for t in range(ntv):
                out[:, n * 8 + t] = v[t * 128:(t + 1) * 128]
        return out

    pw_all = [pw0] + [pw[i] for i in range(14)]
    ab_all = [ab0] + [ab_i[i] for i in range(14)]
    pwh = pack_cols([0.5 * v for v in pw_all], N_NODES * 8)
    pw2 = pack_cols([(-0.5 / B) * v for v in pw_all], N_NODES * 8)
    ab_p = pack_cols(ab_all, N_NODES * 8)
    cb_p = np.zeros((128, N_CALC * 4), np.float32)
    cb_all = [cb0] + [cb[i] for i in range(6)]
    for n, v in enumerate(cb_all):
        for t in range(4):
            cb_p[:, n * 4 + t] = v[t * 128:(t + 1) * 128]

    ohe = np.zeros((N_NODES, 128, 16), np.float32)
    for n in range(N_NODES):
        ohe[n, :, n] = 1.0
    ohe = ohe.astype(_BF)

    leaf_bf = leaf_out.astype(_BF)

    # gate-flip coefficients: alternate rows [q, p]: even -> q = 1 - p
    flip = np.zeros((32, 4), np.float32)
    for i in range(32):
        flip[i, 0] = -1.0 if i % 2 == 0 else 1.0
        flip[i, 1] = 1.0 if i % 2 == 0 else 0.0
        flip[i, 2] = -1.0 if i < 2 else 1.0       # E2 pattern [q0,q0,p0,p0]
        flip[i, 3] = 1.0 if i < 2 else 0.0

    shared = {
        "cw0": cw0, "cw": cw, "aw0": aw0, "aw": aw,
        "pwh": pwh, "pw2": pw2, "ab": ab_p, "cb": cb_p,
        "ohe": ohe, "leaf": leaf_bf, "flip": flip,
    }
    return [dict(shared, x_fm=x_cores[c]) for c in range(N_CORES)]


def get_nc(chain=0):
    key = f"nc{chain}"
    if key not in _CACHE:
        _CACHE[key] = _build(chain=chain)
    return _CACHE[key]


def kernel(**inputs) -> np.ndarray:
    nc = get_nc()
    in_maps = _pack_inputs(inputs)
    res = bass_utils.run_bass_kernel_spmd(nc, in_maps,
                                          core_ids=list(range(N_CORES)))
    return np.concatenate([res.results[c]["out"] for c in range(N_CORES)],
                          axis=0)


# revision 18
# speedup vs baseline: 1.7147x; 1.7147x over previous
"""Trainium2 Bass kernel for nn_DeepDDT (soft decision tree, 16 leaves).

Self-contained: takes FULL unsharded inputs, shards batch across 8 NeuronCores
(pure data parallel), runs a Bass/Tile kernel per core, gathers full output.

Algorithmic restructuring vs the reference:
  - calc matmuls for nodes 7..14 are dead (leaves don't consume outs) -> skipped
  - comp_n = -pw_n * mean_B(inp_n); all batch means are collected locally and
    combined with ONE 16KB AllReduce (the outs-chain itself is collective-free)
  - sigmoid(dist) is replaced by tanh: p = 0.5 + 0.5 * sum(T*e)/sum(e) with
    T = tanh(0.5*(pw*inp + comp)), so tanh/exp/relu all live in one ACT table set
  - softmax max-subtraction is skipped (|z| < 4, exp is safe in fp32)
  - leaf probabilities via a 4-level shuffle/multiply tree on-chip
All data is feat-major on chip ([feature_partition, batch_free]) so per-feature
scale/bias (pw, comp, attn_b, calc_b) are per-partition ACT operands.

chain>1 (used only by test.py for timing): the NEFF executes the body once
with the AllReduce, then `chain` more times inside a hardware For_i loop.
Collectives cannot replay inside a NEFF loop on this runtime (mesh desync),
so the loop iterations reuse the pre-loop AllReduce result; they still issue
the same DRAM round-trip DMAs for the reduction buffers. One launch therefore
performs chain+1 full kernel iterations; the AllReduce itself is timed
separately by test.py and added per-iteration.
"""

import numpy as np
import ml_dtypes

import concourse.bass as bass
import concourse.mybir as mybir
import concourse.tile as tile
from concourse import bacc, bass_utils

AF = mybir.ActivationFunctionType
ALU = mybir.AluOpType
F32 = mybir.dt.float32
BF16 = mybir.dt.bfloat16
FP8 = mybir.dt.float8e4
DR = mybir.MatmulPerfMode.DoubleRow
WS = 64.0                  # fp8 weight scale (descaled via ACT scale=1/WS)

N_CORES = 8
B, D, OUT, LEAF = 8192, 512, 64, 16
BS = B // N_CORES          # 1024 rows per core
F = 2 * D                  # 1024 internal-node input width
NT_X = D // 128            # 4 tiles of x features
NT_F = F // 128            # 8 tiles of concat features
N_NODES = 15               # root + 14 internal
N_CALC = 7                 # nodes with live calc matmuls (0..6)
BH = BS // 2               # 512: batch half (PSUM bank limit)

_BF = ml_dtypes.bfloat16
_F8 = ml_dtypes.float8_e4m3

_CACHE = {}


def _parent(n):
    return (n - 1) // 2


def _build(debug=False, skip_attn=False, skip_tail=False, chain=0):
    assert chain == 0 or not (debug or skip_attn or skip_tail)
    nc = bacc.Bacc("TRN2", target_bir_lowering=False, debug=False,
                   num_devices=N_CORES)

    # ---------------- DRAM I/O ----------------
    x_d = nc.dram_tensor("x_fm", [NT_X, 128, BS], FP8, kind="ExternalInput")
    cw0_d = nc.dram_tensor("cw0", [NT_X, 128, D], FP8, kind="ExternalInput")
    cw_d = nc.dram_tensor("cw", [N_CALC - 1, NT_F, 128, D], FP8, kind="ExternalInput")
    aw0_d = nc.dram_tensor("aw0", [NT_X, 128, D], FP8, kind="ExternalInput")
    aw_d = nc.dram_tensor("aw", [14, NT_F, 128, F], FP8, kind="ExternalInput")
    pwh_d = nc.dram_tensor("pwh", [128, N_NODES * 8], F32, kind="ExternalInput")
    pw2_d = nc.dram_tensor("pw2", [128, N_NODES * 8], F32, kind="ExternalInput")
    ab_d = nc.dram_tensor("ab", [128, N_NODES * 8], F32, kind="ExternalInput")
    cb_d = nc.dram_tensor("cb", [128, N_CALC * 4], F32, kind="ExternalInput")
    ohe_d = nc.dram_tensor("ohe", [N_NODES, 128, 16], BF16, kind="ExternalInput")
    leaf_d = nc.dram_tensor("leaf", [LEAF, OUT], BF16, kind="ExternalInput")
    flip_d = nc.dram_tensor("flip", [32, 4], F32, kind="ExternalInput")
    out_d = nc.dram_tensor("out", [BS, OUT], F32, kind="ExternalOutput")
    if debug:
        dbg_o = nc.dram_tensor("dbg_o", [128, N_CALC * NT_X * BS], BF16,
                               kind="ExternalOutput")
        dbg_ps = nc.dram_tensor("dbg_ps", [128, 32], F32, kind="ExternalOutput")
        dbg_s = nc.dram_tensor("dbg_s", [128, 32], F32, kind="ExternalOutput")
        dbg_bias = nc.dram_tensor("dbg_bias", [128, N_NODES * 8], F32,
                                  kind="ExternalOutput")
        dbg_s1 = nc.dram_tensor("dbg_s1", [16, BS], F32, kind="ExternalOutput")
        dbg_s2 = nc.dram_tensor("dbg_s2", [16, BS], F32, kind="ExternalOutput")
        dbg_pp = nc.dram_tensor("dbg_pp", [32, BS], F32, kind="ExternalOutput")
        dbg_leaf = nc.dram_tensor("dbg_leaf", [32, BS], F32,
                                  kind="ExternalOutput")

    with tile.TileContext(nc) as tc:
        with (
            tc.tile_pool(name="const", bufs=1) as cpool,
            tc.tile_pool(name="obuf", bufs=1) as opool,
            tc.tile_pool(name="wc", bufs=2) as wcpool,
            tc.tile_pool(name="wa", bufs=2) as wapool,
            tc.tile_pool(name="ebuf", bufs=3) as epool,
            tc.tile_pool(name="tbuf", bufs=3) as tpool,
            tc.tile_pool(name="debuf", bufs=3) as depool,
            tc.tile_pool(name="misc", bufs=1) as mpool,
            tc.tile_pool(name="spsum", bufs=1, space="PSUM") as spool,
            tc.tile_pool(name="dram", bufs=1, space="DRAM") as dpool,
        ):
            # ---------- constants ----------
            # x/o/weights live in fp8 (e4m3); weights are pre-scaled by WS on
            # the host and descaled via ACT scale=1/WS after each matmul.
            x_sb = cpool.tile([128, NT_X, BS], FP8)
            for t in range(NT_X):
                nc.gpsimd.dma_start(x_sb[:, t, :], x_d[t])
            pwh = cpool.tile([128, N_NODES * 8], F32)
            nc.gpsimd.dma_start(pwh[:], pwh_d[:])
            pw2 = cpool.tile([128, N_NODES * 8], F32)
            nc.gpsimd.dma_start(pw2[:], pw2_d[:])
            ab = cpool.tile([128, N_NODES * 8], F32)
            nc.gpsimd.dma_start(ab[:], ab_d[:])
            cb = cpool.tile([128, N_CALC * 4], F32)
            nc.gpsimd.dma_start(cb[:], cb_d[:])
            ohe = cpool.tile([128, N_NODES * 16], BF16)
            for n in range(N_NODES):
                nc.gpsimd.dma_start(ohe[:, n * 16:(n + 1) * 16], ohe_d[n])
            leaf_w = cpool.tile([LEAF, OUT], BF16)
            nc.gpsimd.dma_start(leaf_w[:], leaf_d[:])
            flip = cpool.tile([32, 4], F32)
            nc.gpsimd.dma_start(flip[:], flip_d[:])

            # o_sb: calc outputs for nodes 0..6, feat-major fp8
            o_sb = cpool.tile([128, N_CALC * NT_X, BS], FP8)

            def o_tile(c, t):  # feature tile t of node c's output
                return o_sb[:, c * NT_X + t, :]

            def inp_tile(n, t):
                """feature tile t of node n's prob/attn input (feat-major)."""
                if n == 0:
                    return x_sb[:, t, :]
                if t < NT_X:
                    return o_tile(_parent(n), t)
                return x_sb[:, t - NT_X, :]

            def inp_pair(n, j, h):
                """k-tile pair j (tiles 2j,2j+1), batch half h: [128,2,BH] AP
                for a DoubleRow matmul rhs."""
                hs = slice(h * BH, h * BH + BH)
                if n == 0:
                    return x_sb[:, 2 * j:2 * j + 2, hs]
                if j < NT_X // 2:
                    base = _parent(n) * NT_X + 2 * j
                    return o_sb[:, base:base + 2, hs]
                jj = 2 * (j - NT_X // 2)
                return x_sb[:, jj:jj + 2, hs]

            # partial sums tile: cols 0..3 = x tiles, 4+c*4+m = o_c tile m
            ps = mpool.tile([128, 32], F32, tag="ps")

            # s1/s2 accumulators (rows 0..14 = nodes)
            s1 = spool.tile([16, BS], F32, space="PSUM")
            s2 = spool.tile([16, BS], F32, space="PSUM")

            # collective DRAM round-trip buffers (shared by all reps)
            cc_in = dpool.tile([128, 32], F32)
            cc_out = dpool.tile([128, 32], F32)

            def emit_body(rep, with_cc):
                with tc.tile_pool(name=f"zpsum{rep}", bufs=2,
                                  space="PSUM") as zpool:
                    # ================= CALC PHASE =================
                    for t in range(NT_X):
                        nc.vector.reduce_sum(
                            ps[:, t:t + 1], x_sb[:, t, :],
                            axis=mybir.AxisListType.X)
                    for c in range(N_CALC):
                        ntk = NT_X if c == 0 else NT_F
                        wct = wcpool.tile([128, NT_F, D], FP8, tag="wc")
                        for k in range(ntk):
                            src = cw0_d[k] if c == 0 else cw_d[c - 1, k]
                            nc.gpsimd.dma_start(wct[:, k, :], src)
                        for m in range(NT_X):  # output feature tiles (D=512)
                            zp = zpool.tile([128, BS], F32, tag="zp")
                            for j in range(ntk // 2):
                                lhs = wct[:, 2 * j:2 * j + 2,
                                          m * 128:(m + 1) * 128]
                                for h in range(2):
                                    rhs = (x_sb[:, 2 * j:2 * j + 2,
                                                h * BH:h * BH + BH]
                                           if c == 0 else inp_pair(c, j, h))
                                    nc.tensor.matmul(
                                        zp[:, h * BH: h * BH + BH], lhs, rhs,
                                        start=(j == 0), stop=(j == ntk // 2 - 1),
                                        perf_mode=DR)
                            # relu + fp8 store + free-dim sum in one ACT pass
                            nc.scalar.activation(
                                o_tile(c, m), zp[:], AF.Relu,
                                bias=cb[:, c * 4 + m: c * 4 + m + 1],
                                scale=1.0 / WS,
                                accum_out=ps[:, 4 + c * 4 + m: 5 + c * 4 + m])

                    # ============ ALLREDUCE (one, 16KB) ============
                    # Loop reps (with_cc=False) reuse the pre-loop AllReduce
                    # result in cc_out but still do both DRAM DMAs.
                    nc.gpsimd.dma_start(cc_in[:], ps[:])
                    if with_cc:
                        nc.gpsimd.collective_compute(
                            "AllReduce", ALU.add,
                            replica_groups=[list(range(N_CORES))],
                            ins=[cc_in[:]], outs=[cc_out[:]])
                    s_sb = mpool.tile([128, 32], F32, tag="s_sb")
                    nc.gpsimd.dma_start(s_sb[:], cc_out[:])

                    # comp bias: bias_all = pw2 (.) gathered sums
                    tmp = mpool.tile([128, N_NODES * 8], F32, tag="tmp")
                    nc.vector.memset(tmp[:], 0.0)
                    nc.vector.tensor_copy(tmp[:, 0:4], s_sb[:, 0:4])  # root <- x
                    for n in range(1, N_NODES):
                        pc = 4 + _parent(n) * 4
                        nc.vector.tensor_copy(tmp[:, n * 8:n * 8 + 4],
                                              s_sb[:, pc:pc + 4])
                        nc.vector.tensor_copy(tmp[:, n * 8 + 4:n * 8 + 8],
                                              s_sb[:, 0:4])
                    bias_all = mpool.tile([128, N_NODES * 8], F32, tag="bias")
                    nc.vector.tensor_mul(bias_all[:], pw2[:], tmp[:])
                    if debug:
                        nc.gpsimd.dma_start(dbg_o[:], o_sb[:])
                        nc.gpsimd.dma_start(dbg_ps[:], ps[:])
                        nc.gpsimd.dma_start(dbg_s[:], s_sb[:])
                        nc.gpsimd.dma_start(dbg_bias[:], bias_all[:])

                    # ================= ATTN PHASE =================
                    # Per node: accumulate e and T*e across the m feature
                    # tiles on DVE (bf16), then reduce partitions with ONE
                    # one-hot matmul per (value, batch-half) instead of one
                    # per (m, value, batch-half): 60 PE matmuls instead of 464.
                    # The one-hot reduction for node n is emitted AFTER node
                    # n+1's dense matmuls so the PE queue never waits on the
                    # ACT/DVE accumulation chain (software pipelining).
                    def emit_onehot(n, e_acc, de_acc):
                        oh = ohe[:, n * 16:(n + 1) * 16]
                        first = (n == 0)
                        last = (n == N_NODES - 1)
                        for h in range(2):
                            nc.tensor.matmul(
                                s1[:, h * BH: h * BH + BH], oh,
                                e_acc[:, h * BH: h * BH + BH],
                                start=first, stop=last, skip_group_check=True)
                            nc.tensor.matmul(
                                s2[:, h * BH: h * BH + BH], oh,
                                de_acc[:, h * BH: h * BH + BH],
                                start=first, stop=last, skip_group_check=True)

                    pending = None
                    for n in ([] if skip_attn else range(N_NODES)):
                        ntf = NT_X if n == 0 else NT_F
                        wat = wapool.tile([128, NT_F, F], FP8, tag="wa")
                        wid = D if n == 0 else F
                        for k in range(ntf):
                            src = aw0_d[k] if n == 0 else aw_d[n - 1, k]
                            nc.gpsimd.dma_start(wat[:, k, 0:wid], src)
                        e_acc = epool.tile([128, BS], BF16, tag="eacc", bufs=2)
                        de_acc = depool.tile([128, BS], BF16, tag="deacc", bufs=2)
                        for m in range(ntf):
                            zp = zpool.tile([128, BS], F32, tag="zp")
                            for j in range(ntf // 2):
                                lhs = wat[:, 2 * j:2 * j + 2,
                                          m * 128:(m + 1) * 128]
                                for h in range(2):
                                    rhs = inp_pair(n, j, h)
                                    nc.tensor.matmul(
                                        zp[:, h * BH: h * BH + BH], lhs, rhs,
                                        start=(j == 0), stop=(j == ntf // 2 - 1),
                                        perf_mode=DR)
                            col = n * 8 + m
                            e_m = (e_acc if m == 0 else
                                   epool.tile([128, BS], BF16, tag="e"))
                            nc.scalar.activation(e_m[:], zp[:], AF.Exp,
                                                 bias=ab[:, col:col + 1],
                                                 scale=1.0 / WS)
                            t_m = tpool.tile([128, BS], BF16, tag="t")
                            nc.scalar.activation(
                                t_m[:], inp_tile(n, m), AF.Tanh,
                                bias=bias_all[:, col:col + 1],
                                scale=pwh[:, col:col + 1])
                            if m == 0:
                                nc.vector.tensor_mul(de_acc[:], t_m[:], e_m[:])
                            else:
                                de_m = depool.tile([128, BS], BF16, tag="de")
                                nc.vector.tensor_mul(de_m[:], t_m[:], e_m[:])
                                nc.vector.tensor_add(e_acc[:], e_acc[:], e_m[:])
                                nc.vector.tensor_add(de_acc[:], de_acc[:], de_m[:])
                        if pending is not None:
                            emit_onehot(*pending)
                        pending = (n, e_acc, de_acc)
                    if pending is not None:
                        emit_onehot(*pending)

                # ================= TAIL =================
                # Gate selection: row pattern [q_n, p_n] alternating is obtained
                # by shuffling p rows then flipping alternate rows via per-
                # partition affine (q = 1 - p); avoids any non-32-aligned
                # partition access.
                if skip_attn:
                    zt = mpool.tile([128, OUT], F32, tag="zt")
                    nc.vector.memset(zt[:], 0.0)
                    for bt in range(BS // 128):
                        nc.gpsimd.dma_start(out_d[bt * 128:(bt + 1) * 128, :], zt[:])
                    zt16 = mpool.tile([16, BS], F32, tag="zt16")
                    nc.vector.memset(zt16[:], 0.0)
                    nc.vector.tensor_copy(s1[:], zt16[:])
                    nc.vector.tensor_copy(s2[:], zt16[:])
                if skip_tail:
                    if debug:
                        s1c = mpool.tile([16, BS], F32, tag="s1c")
                        nc.vector.tensor_copy(s1c[:], s1[:])
                        nc.gpsimd.dma_start(dbg_s1[:], s1c[:])
                        s2c = mpool.tile([16, BS], F32, tag="s2c")
                        nc.vector.tensor_copy(s2c[:], s2[:])
                        nc.gpsimd.dma_start(dbg_s2[:], s2c[:])
                        zt32 = mpool.tile([32, BS], F32, tag="zt32")
                        nc.vector.memset(zt32[:], 0.0)
                        nc.gpsimd.dma_start(dbg_pp[:], zt32[:])
                        nc.gpsimd.dma_start(dbg_leaf[:], zt32[:])
                    if not skip_attn:
                        zt = mpool.tile([128, OUT], F32, tag="zt")
                        nc.vector.memset(zt[:], 0.0)
                        for bt in range(BS // 128):
                            nc.gpsimd.dma_start(out_d[bt * 128:(bt + 1) * 128, :],
                                                zt[:])
                with tc.tile_pool(name=f"tpsum{rep}", bufs=2,
                                  space="PSUM") as tpsum:
                  if not skip_tail:
                    rec = mpool.tile([16, BS], F32, tag="rec")
                    nc.vector.reciprocal(rec[:], s1[:])
                    rat = mpool.tile([16, BS], F32, tag="rat")
                    nc.vector.tensor_mul(rat[:], s2[:], rec[:])
                    pp = mpool.tile([32, BS], F32, tag="pp")
                    nc.vector.tensor_scalar(pp[0:16, :], rat[:], 0.5, 0.5,
                                            ALU.mult, ALU.add)

                    def shuf(dst, src, mask):
                        mask = mask + [0] * (32 - len(mask))
                        nc.vector.stream_shuffle(dst[:], src[:], mask=mask)

                    # flip coefficient columns: 0=alt_a 1=alt_b 2=pair_a 3=pair_b
                    e2 = mpool.tile([32, BS], F32, tag="e2")
                    s2v = mpool.tile([32, BS], F32, tag="s2v")
                    shuf(e2, pp, [0, 0, 0, 0])
                    nc.vector.tensor_scalar(e2[0:4, :], e2[0:4, :],
                                            flip[0:4, 2:3], flip[0:4, 3:4],
                                            ALU.mult, ALU.add)
                    shuf(s2v, pp, [1, 1, 2, 2])
                    nc.vector.tensor_scalar(s2v[0:4, :], s2v[0:4, :],
                                            flip[0:4, 0:1], flip[0:4, 1:2],
                                            ALU.mult, ALU.add)
                    l2 = mpool.tile([32, BS], F32, tag="l2")
                    nc.vector.tensor_mul(l2[0:4, :], e2[0:4, :], s2v[0:4, :])
                    e3 = mpool.tile([32, BS], F32, tag="e3")
                    s3v = mpool.tile([32, BS], F32, tag="s3v")
                    shuf(e3, l2, [0, 0, 1, 1, 2, 2, 3, 3])
                    shuf(s3v, pp, [3, 3, 4, 4, 5, 5, 6, 6])
                    nc.vector.tensor_scalar(s3v[0:8, :], s3v[0:8, :],
                                            flip[0:8, 0:1], flip[0:8, 1:2],
                                            ALU.mult, ALU.add)
                    l3 = mpool.tile([32, BS], F32, tag="l3")
                    nc.vector.tensor_mul(l3[0:8, :], e3[0:8, :], s3v[0:8, :])
                    e4 = mpool.tile([32, BS], F32, tag="e4")
                    s4v = mpool.tile([32, BS], F32, tag="s4v")
                    shuf(e4, l3, [i // 2 for i in range(16)])
                    shuf(s4v, pp, sum([[7 + i, 7 + i] for i in range(8)], []))
                    nc.vector.tensor_scalar(s4v[0:16, :], s4v[0:16, :],
                                            flip[0:16, 0:1], flip[0:16, 1:2],
                                            ALU.mult, ALU.add)
                    leaf_p = mpool.tile([32, BS], BF16, tag="leaf_p")
                    nc.vector.tensor_mul(leaf_p[0:16, :], e4[0:16, :], s4v[0:16, :])
                    if debug:
                        s1c = mpool.tile([16, BS], F32, tag="s1c")
                        nc.vector.tensor_copy(s1c[:], s1[:])
                        nc.gpsimd.dma_start(dbg_s1[:], s1c[:])
                        s2c = mpool.tile([16, BS], F32, tag="s2c")
                        nc.vector.tensor_copy(s2c[:], s2[:])
                        nc.gpsimd.dma_start(dbg_s2[:], s2c[:])
                        nc.gpsimd.dma_start(dbg_pp[:], pp[:])
                        lpc = mpool.tile([32, BS], F32, tag="lpc")
                        nc.vector.memset(lpc[:], 0.0)
                        nc.vector.tensor_copy(lpc[0:16, :], e4[0:16, :])
                        nc.vector.tensor_mul(lpc[0:16, :], lpc[0:16, :], s4v[0:16, :])
                        nc.gpsimd.dma_start(dbg_leaf[:], lpc[:])

                    # actions + softmax, batch-major
                    for bt in range(BS // 128):
                        ap = tpsum.tile([128, OUT], F32, tag="act")
                        nc.tensor.matmul(ap[:], leaf_p[0:16, bt * 128:(bt + 1) * 128],
                                         leaf_w[:], start=True, stop=True)
                        ea = mpool.tile([128, OUT], F32, tag="ea", bufs=2)
                        nc.scalar.activation(ea[:], ap[:], AF.Exp)
                        ssum = mpool.tile([128, 1], F32, tag="ssum", bufs=2)
                        nc.vector.reduce_sum(ssum[:], ea[:], axis=mybir.AxisListType.X)
                        rs = mpool.tile([128, 1], F32, tag="rs", bufs=2)
                        nc.vector.reciprocal(rs[:], ssum[:])
                        ot = mpool.tile([128, OUT], F32, tag="ot", bufs=2)
                        nc.vector.tensor_scalar(ot[:], ea[:], rs[:], None, ALU.mult)
                        nc.gpsimd.dma_start(out_d[bt * 128:(bt + 1) * 128, :], ot[:])

            emit_body(0, True)
            if chain > 0:
                with tc.For_i(0, chain, staggered_reset=True):
                    emit_body(1, False)

    nc.compile()
    return nc


def _pack_inputs(inputs):
    """Host-side packing: transposes, bf16 casts, per-node packed vectors."""
    x = np.asarray(inputs["x"], np.float32)
    cW0 = np.asarray(inputs["calc_W0"], np.float32)
    cb0 = np.asarray(inputs["calc_b0"], np.float32)
    pw0 = np.asarray(inputs["prob_w0"], np.float32)
    aW0 = np.asarray(inputs["attn_W0"], np.float32)
    ab0 = np.asarray(inputs["attn_b0"], np.float32)
    cW = np.asarray(inputs["calc_W"], np.float32)
    cb = np.asarray(inputs["calc_b"], np.float32)
    pw = np.asarray(inputs["prob_w"], np.float32)
    aW = np.asarray(inputs["attn_W"], np.float32)
    ab_i = np.asarray(inputs["attn_b"], np.float32)
    leaf_out = np.asarray(inputs["leaf_out"], np.float32)

    # x feat-major per core: [NT_X, 128, BS], fp8
    x_fm = np.ascontiguousarray(x.T).astype(_F8)          # [D, B]
    x_cores = [np.ascontiguousarray(
        x_fm[:, c * BS:(c + 1) * BS].reshape(NT_X, 128, BS))
        for c in range(N_CORES)]

    # weights: lhsT = W.T scaled by WS, split into k-tiles [K/128, 128, M], fp8
    def kt(wT, m):  # wT [K, M]
        return np.ascontiguousarray((wT * WS).reshape(-1, 128, m)).astype(_F8)

    cw0 = kt(cW0.T, D)                                    # [4,128,512]
    cw = np.stack([kt(cW[i].T, D) for i in range(6)])     # [6,8,128,512]
    aw0 = kt(aW0.T, D)                                    # [4,128,512]
    aw = np.stack([kt(aW[i].T, F) for i in range(14)])    # [14,8,128,1024]

    def pack_cols(vecs, ncols):
        """vecs: list of (per-node 1D arrays); -> [128, ncols]"""
        out = np.zeros((128, ncols), np.float32)
        for n, v in enumerate(vecs):
            ntv = v.shape[0] // 128
            for t in range(ntv):
                out[:, n * 8 + t] = v[t * 128:(t + 1) * 128]
        return out

    pw_all = [pw0] + [pw[i] for i in range(14)]
    ab_all = [ab0] + [ab_i[i] for i in range(14)]
    pwh = pack_cols([0.5 * v for v in pw_all], N_NODES * 8)
    pw2 = pack_cols([(-0.5 / B) * v for v in pw_all], N_NODES * 8)
    ab_p = pack_cols(ab_all, N_NODES * 8)
    cb_p = np.zeros((128, N_CALC * 4), np.float32)
    cb_all = [cb0] + [cb[i] for i in range(6)]
    for n, v in enumerate(cb_all):
        for t in range(4):
            cb_p[:, n * 4 + t] = v[t * 128:(t + 1) * 128]

    ohe = np.zeros((N_NODES, 128, 16), np.float32)
    for n in range(N_NODES):
        ohe[n, :, n] = 1.0
    ohe = ohe.astype(_BF)

    leaf_bf = leaf_out.astype(_BF)

    # gate-flip coefficients: alternate rows [q, p]: even -> q = 1 - p
    flip = np.zeros((32, 4), np.float32)
    for i in range(32):
        flip[i, 0] = -1.0 if i % 2 == 0 else 1.0
        flip[i, 1] = 1.0 if i % 2 == 0 else 0.0
        flip[i, 2] = -1.0 if i < 2 else 1.0       # E2 pattern [q0,q0,p0,p0]
        flip[i, 3] = 1.0 if i < 2 else 0.0

    shared = {
        "cw0": cw0, "cw": cw, "aw0": aw0, "aw": aw,
        "pwh": pwh, "pw2": pw2, "ab": ab_p, "cb": cb_p,
        "ohe": ohe, "leaf": leaf_bf, "flip": flip,
    }
    return [dict(shared, x_fm=x_cores[c]) for c in range(N_CORES)]


def get_nc(chain=0):
    key = f"nc{chain}"
    if key not in _CACHE:
        _CACHE[key] = _build(chain=chain)
    return _CACHE[key]


def kernel(**inputs) -> np.ndarray:
    nc = get_nc()
    in_maps = _pack_inputs(inputs)
    res = bass_utils.run_bass_kernel_spmd(nc, in_maps,
                                          core_ids=list(range(N_CORES)))
    return np.concatenate([res.results[c]["out"] for c in range(N_CORES)],
                          axis=0)
